# revision 12
# baseline (speedup 1.0000x reference)
"""Causal self-attention on 8 TRN2 NeuronCores (Bass/Tile, bf16).

Sharding: core c = 4*bp + hg handles batches [2bp, 2bp+1] and heads
[4hg, 4hg+4). Host transposes x to [B, D, S] bf16, slices weights per
head group; cores emit y^T [D, S] bf16 partials which the host sums
over the 4 head groups and transposes.

Per-core kernel, per batch (phases FUSED chunk-wise so the ACT engine
never idles during QKV and the PE never idles during attention):
  for ch in 0..3:
    A) QKV projection for token chunk ch: q/k in pair-packed
       [2x64 hd, token] layout (2 heads per 128-partition tile), v in
       [token, hd+1] layout with a ones column (softmax denominators
       fall out of the AV matmul).
    B) attention sections (pair g, i-tile it=ch) - these need only
       k/v chunks <= ch. Per section the j-loop computes scores^T for
       BOTH heads concurrently via row-tiled K=64 matmuls
       (tile_position (0,0)/(64,0)), one exp over [128, 2*512] on ACT,
       a triangle mask multiply on the 128-col diagonal band only, and
       per-head AV accumulation with causal column trimming; AV(jt) is
       emitted after scores(jt+1) (1-block software pipeline skew).
    C) output projection chunk of the PREVIOUS batch (wo stationary).
PSUM: scores 2x2 banks, AV accumulators 2x1, qkv/proj aux 2x1 = 8.
"""
import numpy as np

B, S, D, H = 4, 2048, 1024, 16
HD = D // H            # 64
SCALE = 1.0 / np.sqrt(HD)
NB = 2                 # batches per core
NHC = 4                # heads per core (2 pairs)
HCOLS = NHC * HD       # 256 q/k/v columns per core
NDT = D // 128         # 8 D-tiles
NIT = S // 512         # 4 i-tiles per batch
NJT = S // 128         # 16 j-tiles per batch

_NC = None
LAST_RESULT = None


def _build():
    import concourse.bacc as bacc
    import concourse.mybir as mybir
    import concourse.tile as tile
    import concourse.bass as bass

    f32 = mybir.dt.float32
    bf16 = mybir.dt.bfloat16
    Act = mybir.ActivationFunctionType

    nc = bacc.Bacc(trn_type="TRN2", target_bir_lowering=False)
    xT = nc.dram_tensor("xT", [NB, D, S], bf16, kind="ExternalInput")
    wqk = nc.dram_tensor("wqk", [D, 2 * HCOLS], bf16, kind="ExternalInput")
    wv = nc.dram_tensor("wv", [D, HCOLS], bf16, kind="ExternalInput")
    wo = nc.dram_tensor("wo", [HCOLS, D], bf16, kind="ExternalInput")
    bqk = nc.dram_tensor("bqk", [2 * HCOLS], f32, kind="ExternalInput")
    bv = nc.dram_tensor("bv", [HCOLS], f32, kind="ExternalInput")
    mask = nc.dram_tensor("mask", [128, 128], bf16, kind="ExternalInput")
    yT = nc.dram_tensor("yT", [NB, D, S], bf16, kind="ExternalOutput")

    with tile.TileContext(nc) as tc:
        with (
            tc.tile_pool(name="singles", bufs=1) as singles,
            tc.tile_pool(name="xtp", bufs=2) as xtp,
            tc.tile_pool(name="qkp", bufs=2) as qkp,
            tc.tile_pool(name="vp", bufs=2) as vp,
            tc.tile_pool(name="attp", bufs=4) as attp,
            tc.tile_pool(name="yhp", bufs=2) as yhp,
            tc.tile_pool(name="stgp", bufs=4) as stgp,
            tc.tile_pool(name="bcp", bufs=2) as bcp,
            tc.tile_pool(name="pkp", bufs=2) as pkp,
            tc.tile_pool(name="outp", bufs=4) as outp,
            tc.tile_pool(name="dscr", bufs=2, space="DRAM") as dscr,
            tc.tile_pool(name="psS", bufs=2, space="PSUM") as psS,
            tc.tile_pool(name="psA", bufs=2, space="PSUM") as psA,
            tc.tile_pool(name="psX", bufs=2, space="PSUM") as psX,
        ):
            # ---- one-time loads ----
            wqk_sb = singles.tile([128, NDT, 2 * HCOLS], bf16, tag="wqk")
            wqk_r = wqk.ap().rearrange("(dt p) c -> p dt c", p=128)
            for dt in range(NDT):  # split so the first matmul starts early
                nc.sync.dma_start(out=wqk_sb[:, dt, :], in_=wqk_r[:, dt, :])
            wv_sb = singles.tile([128, NDT, HCOLS], bf16, tag="wv")
            nc.sync.dma_start(
                out=wv_sb, in_=wv.ap().rearrange("(dt p) c -> p dt c", p=128)
            )
            wo_sb = singles.tile([128, 2, D], bf16, tag="wo")
            nc.sync.dma_start(
                out=wo_sb, in_=wo.ap().rearrange("(kt p) c -> p kt c", p=128)
            )
            bqk_sb = singles.tile([128, 4], f32, tag="bqk")
            nc.sync.dma_start(
                out=bqk_sb, in_=bqk.ap().rearrange("(cb p) -> p cb", p=128)
            )
            bv_sb = singles.tile([128, HCOLS], f32, tag="bv")
            bv_ap = bv.ap()
            nc.gpsimd.dma_start(
                out=bv_sb,
                in_=bass.AP(
                    tensor=bv_ap.tensor, offset=bv_ap.offset,
                    ap=[[0, 128], *bv_ap.ap],
                ),
            )
            mask_sb = singles.tile([128, 128], bf16, tag="mask")
            nc.sync.dma_start(out=mask_sb, in_=mask.ap())
            ones_sb = singles.tile([128, NJT * NHC], bf16, tag="ones")
            nc.vector.memset(ones_sb[:], 1.0)

            def scoped(name, fn):
                s = nc.enter_named_scope(name, False)
                fn()
                nc.leave_named_scope(name, s[0], False)

            def emit_qkv_chunk(b, ch, qp, kT, v_sb):
                t0 = ch * 512
                xt = xtp.tile([128, NDT, 512], bf16, tag="xt")
                for dt in range(NDT):
                    nc.sync.dma_start(
                        out=xt[:, dt, :],
                        in_=xT.ap()[b, dt * 128 : (dt + 1) * 128, t0 : t0 + 512],
                    )
                for cb in range(4):  # q-pair0 q-pair1 k-pair0 k-pair1
                    ps = psX.tile([128, 512], f32, tag="aux",
                                  name=f"psqk_{b}_{ch}_{cb}")
                    for dt in range(NDT):
                        nc.tensor.matmul(
                            ps[:],
                            wqk_sb[:, dt, cb * 128 : (cb + 1) * 128],
                            xt[:, dt, :],
                            start=(dt == 0), stop=(dt == NDT - 1),
                        )
                    dst = qp[cb] if cb < 2 else kT[cb - 2]
                    nc.vector.tensor_scalar_add(
                        out=dst[:, t0 : t0 + 512],
                        in0=ps[:],
                        scalar1=bqk_sb[:, cb : cb + 1],
                    )
                for st in range(4):  # 128-token tiles for v
                    psv = psX.tile([128, 512], f32, tag="aux",
                                   name=f"psv_{b}_{ch}_{st}")
                    for dt in range(NDT):
                        nc.tensor.matmul(
                            psv[:, 0:HCOLS],
                            xt[:, dt, st * 128 : (st + 1) * 128],
                            wv_sb[:, dt, :],
                            start=(dt == 0), stop=(dt == NDT - 1),
                        )
                    nc.vector.tensor_add(
                        v_sb[:, ch * 4 + st, :, 0:HD],
                        psv[:, 0:HCOLS].rearrange("p (h c) -> p h c", h=NHC),
                        bv_sb[:].rearrange("p (h c) -> p h c", h=NHC),
                    )

            def emit_attn_section(b, g, it, qp, kT, v_sb, yh):
                acc = [
                    psA.tile([HD + 1, 512], f32, tag="acc",
                             name=f"acc_{b}_{g}_{it}_{hh}")
                    for hh in range(2)
                ]
                n_jt = 4 * it + 4
                atts = {}

                def emit_scores(jt):
                    diag = jt >= 4 * it
                    r = jt - 4 * it if diag else 0
                    w = 512 - 128 * r
                    pss = psS.tile([128, 2, 512], f32, tag="sc",
                                   name=f"pss_{b}_{g}_{it}_{jt}")
                    for hh in range(2):  # concurrent row-tiled pair
                        nc.tensor.matmul(
                            pss[:, hh, 128 * r : 512],
                            kT[g][64 * hh : 64 * hh + 64,
                                  jt * 128 : (jt + 1) * 128],
                            qp[g][64 * hh : 64 * hh + 64,
                                  it * 512 + 128 * r : (it + 1) * 512],
                            start=True, stop=True,
                        )
                    att = attp.tile([128, 2, 512], bf16, tag="att")
                    nc.scalar.activation(
                        out=att[:, :, 0:w],
                        in_=pss[:, :, 128 * r : 512],
                        func=Act.Exp, bias=0.0, scale=float(SCALE),
                    )
                    if diag:
                        for hh in range(2):
                            nc.vector.tensor_mul(
                                att[:, hh, 0:128], att[:, hh, 0:128],
                                mask_sb[:],
                            )
                    atts[jt] = att

                def emit_av(jt):
                    diag = jt >= 4 * it
                    r = jt - 4 * it if diag else 0
                    w = 512 - 128 * r
                    att = atts.pop(jt)
                    for hh in range(2):
                        nc.tensor.matmul(
                            acc[hh][:, 128 * r : 512],
                            v_sb[:, jt, 2 * g + hh, :],
                            att[:, hh, 0:w],
                            start=(jt == 0), stop=(jt == n_jt - 1),
                        )

                # 1-block software-pipeline skew
                for jt in range(n_jt + 1):
                    if jt < n_jt:
                        emit_scores(jt)
                    if jt >= 1:
                        emit_av(jt - 1)

                # section drain: denominators + stage + normalize
                dh = dscr.tile([2, 512], bf16, tag="dh",
                               name=f"dh_{b}_{g}_{it}")
                stgs = []
                for hh in range(2):
                    st_t = stgp.tile([HD + 1, 512], bf16, tag="stg",
                                     name=f"stg_{b}_{g}_{it}_{hh}")
                    nc.vector.tensor_copy(st_t[:], acc[hh][:])
                    nc.sync.dma_start(out=dh[hh, :], in_=st_t[HD : HD + 1, :])
                    stgs.append(st_t)
                pk = pkp.tile([128, 8], bf16, tag="pk", name=f"pk_{b}_{g}_{it}")
                nc.sync.dma_start(
                    out=pk[:],
                    in_=bass.AP(tensor=dh.tensor, offset=dh.offset,
                                ap=[[8, 128], [1, 8]]),
                )
                rec = pkp.tile([128, 8], bf16, tag="rec",
                               name=f"rec_{b}_{g}_{it}")
                with nc.allow_low_precision(
                    reason="bf16 softmax denominators within 2e-2 tolerance"
                ):
                    nc.vector.reciprocal(rec[:], pk[:])
                drec = dscr.tile([2, 512], bf16, tag="drec",
                                 name=f"drec_{b}_{g}_{it}")
                nc.sync.dma_start(
                    out=bass.AP(tensor=drec.tensor, offset=drec.offset,
                                ap=[[8, 128], [1, 8]]),
                    in_=rec[:],
                )
                bch = bcp.tile([64, 2, 512], bf16, tag="bch",
                               name=f"bch_{b}_{g}_{it}")
                nc.gpsimd.dma_start(
                    out=bch[:],
                    in_=bass.AP(tensor=drec.tensor, offset=drec.offset,
                                ap=[[0, 64], [1, 1024]]),
                )
                for hh in range(2):
                    nc.vector.tensor_mul(
                        yh[g][64 * hh : 64 * hh + 64,
                              it * 512 : (it + 1) * 512],
                        stgs[hh][0:HD, :],
                        bch[:, hh, :],
                    )

            def emit_proj_chunk(b, tt, yh):
                for dc in range(NDT):
                    pso = psX.tile([128, 512], f32, tag="aux",
                                   name=f"pso_{b}_{tt}_{dc}")
                    for kt in range(2):
                        nc.tensor.matmul(
                            pso[:],
                            wo_sb[:, kt, dc * 128 : (dc + 1) * 128],
                            yh[kt][:, tt * 512 : (tt + 1) * 512],
                            start=(kt == 0), stop=(kt == 1),
                        )
                    yo = outp.tile([128, 512], bf16, tag="yo")
                    nc.vector.tensor_copy(yo[:], pso[:])
                    nc.gpsimd.dma_start(
                        out=yT.ap()[b, dc * 128 : (dc + 1) * 128,
                                    tt * 512 : (tt + 1) * 512],
                        in_=yo[:],
                    )

            # Per-batch tiles for both batches up front.
            bt = []
            for b in range(NB):
                qp = [
                    qkp.tile([128, S], bf16, tag=f"qp{g}", name=f"qp{g}_{b}")
                    for g in range(2)
                ]
                kT = [
                    qkp.tile([128, S], bf16, tag=f"kT{g}", name=f"kT{g}_{b}")
                    for g in range(2)
                ]
                v_sb = vp.tile([128, NJT, NHC, HD + 1], bf16, tag="v",
                               name=f"v_{b}")
                yh = [
                    yhp.tile([128, S], bf16, tag=f"yh{g}", name=f"yh{g}_{b}")
                    for g in range(2)
                ]
                bt.append((qp, kT, v_sb, yh))

            def emit_ones(b):
                nc.vector.tensor_copy(
                    bt[b][2][:, :, :, HD : HD + 1],
                    ones_sb[:].rearrange("p (a b c) -> p a b c", a=NJT, b=NHC),
                )

            def qkv(b, ch):
                if ch == 0:
                    emit_ones(b)
                scoped(f"qkv{b}", lambda: emit_qkv_chunk(
                    b, ch, bt[b][0], bt[b][1], bt[b][2]))

            def attn(b, it):
                for g in range(2):
                    scoped(f"attn{b}", lambda g=g: emit_attn_section(
                        b, g, it, bt[b][0], bt[b][1], bt[b][2], bt[b][3]))

            def proj(b, tt):
                scoped(f"proj{b}", lambda: emit_proj_chunk(b, tt, bt[b][3]))

            # Emission order = scheduler priority. Attention sections come
            # BEFORE the qkv/proj work of the same round so scores matmuls
            # win ties on the PE and the ACT engine never starves; qkv/proj
            # fill the PE slack. qkv(b,it+1) is emitted after attn(b,it);
            # qkv(b+1,0) rides in batch b's last (ACT-heaviest) round.
            qkv(0, 0)
            for it in range(NIT):
                attn(0, it)
                qkv(0, it + 1) if it < 3 else qkv(1, 0)
            for it in range(NIT):
                attn(1, it)
                if it < 3:
                    qkv(1, it + 1)
                proj(0, it)
                if it > 0:
                    proj(1, it - 1)
            proj(1, 3)

    nc.compile()
    return nc


def _get_nc():
    global _NC
    if _NC is None:
        _NC = _build()
    return _NC


def kernel(x, w_qkv, b_qkv, w_proj, b_proj):
    global LAST_RESULT
    import ml_dtypes
    from concourse.bass_utils import run_bass_kernel_spmd

    bf16 = ml_dtypes.bfloat16
    x = np.asarray(x, dtype=np.float32)
    w_qkv = np.asarray(w_qkv, dtype=np.float32)
    b_qkv = np.asarray(b_qkv, dtype=np.float32)
    w_proj = np.asarray(w_proj, dtype=np.float32)
    b_proj = np.asarray(b_proj, dtype=np.float32)

    xTb = np.ascontiguousarray(x.transpose(0, 2, 1)).astype(bf16)  # [B, D, S]

    jj = np.arange(128)[:, None]
    ii = np.arange(128)[None, :]
    mask = (jj <= ii).astype(bf16)  # triangle band for diagonal blocks

    in_maps = []
    for c in range(8):
        bp, hg = c // 4, c % 4
        cols = slice(hg * HCOLS, (hg + 1) * HCOLS)
        w_q = w_qkv[:, 0:D][:, cols]
        w_k = w_qkv[:, D : 2 * D][:, cols]
        w_v = w_qkv[:, 2 * D : 3 * D][:, cols]
        in_maps.append({
            "xT": np.ascontiguousarray(xTb[2 * bp : 2 * bp + 2]),
            "wqk": np.concatenate([w_q, w_k], axis=1).astype(bf16),
            "wv": np.ascontiguousarray(w_v).astype(bf16),
            "wo": np.ascontiguousarray(w_proj[cols, :]).astype(bf16),
            "bqk": np.ascontiguousarray(
                np.concatenate([b_qkv[0:D][cols], b_qkv[D : 2 * D][cols]])
            ),
            "bv": np.ascontiguousarray(b_qkv[2 * D : 3 * D][cols]),
            "mask": mask,
        })

    nc = _get_nc()
    res = run_bass_kernel_spmd(nc, in_maps, core_ids=list(range(8)))
    LAST_RESULT = res

    out = np.zeros((B, S, D), dtype=np.float32)
    for c in range(8):
        bp = c // 4
        yTc = res.results[c]["yT"]  # [2, D, S] bf16 partial
        for bi in range(2):
            out[2 * bp + bi] += np.asarray(yTc[bi], dtype=np.float32).T
    out += b_proj[None, None, :]
    return out


# revision 14
# speedup vs baseline: 1.0081x; 1.0081x over previous
"""Causal self-attention on 8 TRN2 NeuronCores (Bass/Tile, bf16).

Sharding: core c = 4*bp + hg handles batches [2bp, 2bp+1] and heads
[4hg, 4hg+4). Host transposes x to [B, D, S] bf16, slices weights per
head group; cores emit y^T [D, S] bf16 partials which the host sums
over the 4 head groups and transposes.

Per-core kernel, per batch (phases FUSED chunk-wise so the ACT engine
never idles during QKV and the PE never idles during attention):
  for ch in 0..3:
    A) QKV projection for token chunk ch: q/k in pair-packed
       [2x64 hd, token] layout (2 heads per 128-partition tile), v in
       [token, hd+1] layout with a ones column (softmax denominators
       fall out of the AV matmul).
    B) attention sections (pair g, i-tile it=ch) - these need only
       k/v chunks <= ch. Per section the j-loop computes scores^T for
       BOTH heads concurrently via row-tiled K=64 matmuls
       (tile_position (0,0)/(64,0)), one exp over [128, 2*512] on ACT,
       a triangle mask multiply on the 128-col diagonal band only, and
       per-head AV accumulation with causal column trimming; AV(jt) is
       emitted after scores(jt+1) (1-block software pipeline skew).
    C) output projection chunk of the PREVIOUS batch (wo stationary).
PSUM: scores 2x2 banks, AV accumulators 2x1, qkv/proj aux 2x1 = 8.
"""
import numpy as np

B, S, D, H = 4, 2048, 1024, 16
HD = D // H            # 64
SCALE = 1.0 / np.sqrt(HD)
NB = 2                 # batches per core
NHC = 4                # heads per core (2 pairs)
HCOLS = NHC * HD       # 256 q/k/v columns per core
NDT = D // 128         # 8 D-tiles
NIT = S // 512         # 4 i-tiles per batch
NJT = S // 128         # 16 j-tiles per batch

_NC = None
LAST_RESULT = None


def _build():
    import concourse.bacc as bacc
    import concourse.mybir as mybir
    import concourse.tile as tile
    import concourse.bass as bass

    f32 = mybir.dt.float32
    bf16 = mybir.dt.bfloat16
    Act = mybir.ActivationFunctionType

    nc = bacc.Bacc(trn_type="TRN2", target_bir_lowering=False)
    xT = nc.dram_tensor("xT", [NB, D, S], bf16, kind="ExternalInput")
    wqk = nc.dram_tensor("wqk", [D, 2 * HCOLS], bf16, kind="ExternalInput")
    wv = nc.dram_tensor("wv", [D, HCOLS], bf16, kind="ExternalInput")
    wo = nc.dram_tensor("wo", [HCOLS, D], bf16, kind="ExternalInput")
    bqk = nc.dram_tensor("bqk", [2 * HCOLS], f32, kind="ExternalInput")
    bv = nc.dram_tensor("bv", [HCOLS], f32, kind="ExternalInput")
    mask = nc.dram_tensor("mask", [128, 128], bf16, kind="ExternalInput")
    yT = nc.dram_tensor("yT", [NB, D, S], bf16, kind="ExternalOutput")

    with tile.TileContext(nc) as tc:
        with (
            tc.tile_pool(name="singles", bufs=1) as singles,
            tc.tile_pool(name="xtp", bufs=2) as xtp,
            tc.tile_pool(name="qkp", bufs=2) as qkp,
            tc.tile_pool(name="vp", bufs=2) as vp,
            tc.tile_pool(name="attp", bufs=4) as attp,
            tc.tile_pool(name="yhp", bufs=2) as yhp,
            tc.tile_pool(name="stgp", bufs=4) as stgp,
            tc.tile_pool(name="bcp", bufs=2) as bcp,
            tc.tile_pool(name="pkp", bufs=2) as pkp,
            tc.tile_pool(name="outp", bufs=4) as outp,
            tc.tile_pool(name="dscr", bufs=2, space="DRAM") as dscr,
            tc.tile_pool(name="psS", bufs=2, space="PSUM") as psS,
            tc.tile_pool(name="psA", bufs=2, space="PSUM") as psA,
            tc.tile_pool(name="psX", bufs=2, space="PSUM") as psX,
        ):
            # ---- one-time loads ----
            # Startup-latency-critical loads: interleave wqk dt-slices with
            # the first x chunk's dt-slices on the SP queue so matmul dt=0
            # can begin ~2us in; everything else rides the ACT hardware-DGE
            # queue in parallel.
            wqk_sb = singles.tile([128, NDT, 2 * HCOLS], bf16, tag="wqk")
            wqk_r = wqk.ap().rearrange("(dt p) c -> p dt c", p=128)
            xt0 = xtp.tile([128, NDT, 512], bf16, tag="xt", name="xt_0_0")
            for dt in range(NDT):
                nc.sync.dma_start(out=wqk_sb[:, dt, :], in_=wqk_r[:, dt, :])
                nc.sync.dma_start(
                    out=xt0[:, dt, :],
                    in_=xT.ap()[0, dt * 128 : (dt + 1) * 128, 0:512],
                )
            wv_sb = singles.tile([128, NDT, HCOLS], bf16, tag="wv")
            nc.scalar.dma_start(
                out=wv_sb, in_=wv.ap().rearrange("(dt p) c -> p dt c", p=128)
            )
            bqk_sb = singles.tile([128, 4], f32, tag="bqk")
            nc.scalar.dma_start(
                out=bqk_sb, in_=bqk.ap().rearrange("(cb p) -> p cb", p=128)
            )
            mask_sb = singles.tile([128, 128], bf16, tag="mask")
            nc.scalar.dma_start(out=mask_sb, in_=mask.ap())
            wo_sb = singles.tile([128, 2, D], bf16, tag="wo")
            nc.scalar.dma_start(
                out=wo_sb, in_=wo.ap().rearrange("(kt p) c -> p kt c", p=128)
            )
            bv_sb = singles.tile([128, HCOLS], f32, tag="bv")
            bv_ap = bv.ap()
            nc.gpsimd.dma_start(
                out=bv_sb,
                in_=bass.AP(
                    tensor=bv_ap.tensor, offset=bv_ap.offset,
                    ap=[[0, 128], *bv_ap.ap],
                ),
            )
            ones_sb = singles.tile([128, NJT * NHC], bf16, tag="ones")
            nc.vector.memset(ones_sb[:], 1.0)

            def scoped(name, fn):
                s = nc.enter_named_scope(name, False)
                fn()
                nc.leave_named_scope(name, s[0], False)

            def emit_qkv_chunk(b, ch, qp, kT, v_sb):
                t0 = ch * 512
                if b == 0 and ch == 0:
                    xt = xt0  # loaded during startup
                else:
                    xt = xtp.tile([128, NDT, 512], bf16, tag="xt")
                    nc.sync.dma_start(
                        out=xt[:],
                        in_=xT.ap()[b, :, t0 : t0 + 512].rearrange(
                            "(dt p) s -> p dt s", p=128
                        ),
                    )
                for cb in range(4):  # q-pair0 q-pair1 k-pair0 k-pair1
                    ps = psX.tile([128, 512], f32, tag="aux",
                                  name=f"psqk_{b}_{ch}_{cb}")
                    for dt in range(NDT):
                        nc.tensor.matmul(
                            ps[:],
                            wqk_sb[:, dt, cb * 128 : (cb + 1) * 128],
                            xt[:, dt, :],
                            start=(dt == 0), stop=(dt == NDT - 1),
                        )
                    dst = qp[cb] if cb < 2 else kT[cb - 2]
                    nc.vector.tensor_scalar_add(
                        out=dst[:, t0 : t0 + 512],
                        in0=ps[:],
                        scalar1=bqk_sb[:, cb : cb + 1],
                    )
                for st in range(4):  # 128-token tiles for v
                    psv = psX.tile([128, 512], f32, tag="aux",
                                   name=f"psv_{b}_{ch}_{st}")
                    for dt in range(NDT):
                        nc.tensor.matmul(
                            psv[:, 0:HCOLS],
                            xt[:, dt, st * 128 : (st + 1) * 128],
                            wv_sb[:, dt, :],
                            start=(dt == 0), stop=(dt == NDT - 1),
                        )
                    nc.vector.tensor_add(
                        v_sb[:, ch * 4 + st, :, 0:HD],
                        psv[:, 0:HCOLS].rearrange("p (h c) -> p h c", h=NHC),
                        bv_sb[:].rearrange("p (h c) -> p h c", h=NHC),
                    )

            def emit_attn_section(b, g, it, qp, kT, v_sb, yh):
                acc = [
                    psA.tile([HD + 1, 512], f32, tag="acc",
                             name=f"acc_{b}_{g}_{it}_{hh}")
                    for hh in range(2)
                ]
                n_jt = 4 * it + 4
                atts = {}

                def emit_scores(jt):
                    diag = jt >= 4 * it
                    r = jt - 4 * it if diag else 0
                    w = 512 - 128 * r
                    pss = psS.tile([128, 2, 512], f32, tag="sc",
                                   name=f"pss_{b}_{g}_{it}_{jt}")
                    for hh in range(2):  # concurrent row-tiled pair
                        nc.tensor.matmul(
                            pss[:, hh, 128 * r : 512],
                            kT[g][64 * hh : 64 * hh + 64,
                                  jt * 128 : (jt + 1) * 128],
                            qp[g][64 * hh : 64 * hh + 64,
                                  it * 512 + 128 * r : (it + 1) * 512],
                            start=True, stop=True,
                        )
                    att = attp.tile([128, 2, 512], bf16, tag="att")
                    nc.scalar.activation(
                        out=att[:, :, 0:w],
                        in_=pss[:, :, 128 * r : 512],
                        func=Act.Exp, bias=0.0, scale=float(SCALE),
                    )
                    if diag:
                        for hh in range(2):
                            nc.vector.tensor_mul(
                                att[:, hh, 0:128], att[:, hh, 0:128],
                                mask_sb[:],
                            )
                    atts[jt] = att

                def emit_av(jt):
                    diag = jt >= 4 * it
                    r = jt - 4 * it if diag else 0
                    w = 512 - 128 * r
                    att = atts.pop(jt)
                    for hh in range(2):
                        nc.tensor.matmul(
                            acc[hh][:, 128 * r : 512],
                            v_sb[:, jt, 2 * g + hh, :],
                            att[:, hh, 0:w],
                            start=(jt == 0), stop=(jt == n_jt - 1),
                        )

                # 1-block software-pipeline skew
                for jt in range(n_jt + 1):
                    if jt < n_jt:
                        emit_scores(jt)
                    if jt >= 1:
                        emit_av(jt - 1)

                # section drain: denominators + stage + normalize
                dh = dscr.tile([2, 512], bf16, tag="dh",
                               name=f"dh_{b}_{g}_{it}")
                stgs = []
                for hh in range(2):
                    st_t = stgp.tile([HD + 1, 512], bf16, tag="stg",
                                     name=f"stg_{b}_{g}_{it}_{hh}")
                    nc.vector.tensor_copy(st_t[:], acc[hh][:])
                    nc.sync.dma_start(out=dh[hh, :], in_=st_t[HD : HD + 1, :])
                    stgs.append(st_t)
                pk = pkp.tile([128, 8], bf16, tag="pk", name=f"pk_{b}_{g}_{it}")
                nc.sync.dma_start(
                    out=pk[:],
                    in_=bass.AP(tensor=dh.tensor, offset=dh.offset,
                                ap=[[8, 128], [1, 8]]),
                )
                rec = pkp.tile([128, 8], bf16, tag="rec",
                               name=f"rec_{b}_{g}_{it}")
                with nc.allow_low_precision(
                    reason="bf16 softmax denominators within 2e-2 tolerance"
                ):
                    nc.vector.reciprocal(rec[:], pk[:])
                drec = dscr.tile([2, 512], bf16, tag="drec",
                                 name=f"drec_{b}_{g}_{it}")
                nc.sync.dma_start(
                    out=bass.AP(tensor=drec.tensor, offset=drec.offset,
                                ap=[[8, 128], [1, 8]]),
                    in_=rec[:],
                )
                bch = bcp.tile([64, 2, 512], bf16, tag="bch",
                               name=f"bch_{b}_{g}_{it}")
                nc.gpsimd.dma_start(
                    out=bch[:],
                    in_=bass.AP(tensor=drec.tensor, offset=drec.offset,
                                ap=[[0, 64], [1, 1024]]),
                )
                for hh in range(2):
                    nc.vector.tensor_mul(
                        yh[g][64 * hh : 64 * hh + 64,
                              it * 512 : (it + 1) * 512],
                        stgs[hh][0:HD, :],
                        bch[:, hh, :],
                    )

            def emit_proj_chunk(b, tt, yh):
                for dc in range(NDT):
                    pso = psX.tile([128, 512], f32, tag="aux",
                                   name=f"pso_{b}_{tt}_{dc}")
                    for kt in range(2):
                        nc.tensor.matmul(
                            pso[:],
                            wo_sb[:, kt, dc * 128 : (dc + 1) * 128],
                            yh[kt][:, tt * 512 : (tt + 1) * 512],
                            start=(kt == 0), stop=(kt == 1),
                        )
                    yo = outp.tile([128, 512], bf16, tag="yo")
                    nc.vector.tensor_copy(yo[:], pso[:])
                    nc.gpsimd.dma_start(
                        out=yT.ap()[b, dc * 128 : (dc + 1) * 128,
                                    tt * 512 : (tt + 1) * 512],
                        in_=yo[:],
                    )

            # Per-batch tiles for both batches up front.
            bt = []
            for b in range(NB):
                qp = [
                    qkp.tile([128, S], bf16, tag=f"qp{g}", name=f"qp{g}_{b}")
                    for g in range(2)
                ]
                kT = [
                    qkp.tile([128, S], bf16, tag=f"kT{g}", name=f"kT{g}_{b}")
                    for g in range(2)
                ]
                v_sb = vp.tile([128, NJT, NHC, HD + 1], bf16, tag="v",
                               name=f"v_{b}")
                yh = [
                    yhp.tile([128, S], bf16, tag=f"yh{g}", name=f"yh{g}_{b}")
                    for g in range(2)
                ]
                bt.append((qp, kT, v_sb, yh))

            def emit_ones(b):
                nc.vector.tensor_copy(
                    bt[b][2][:, :, :, HD : HD + 1],
                    ones_sb[:].rearrange("p (a b c) -> p a b c", a=NJT, b=NHC),
                )

            def qkv(b, ch):
                if ch == 0:
                    emit_ones(b)
                scoped(f"qkv{b}", lambda: emit_qkv_chunk(
                    b, ch, bt[b][0], bt[b][1], bt[b][2]))

            def attn(b, it):
                for g in range(2):
                    scoped(f"attn{b}", lambda g=g: emit_attn_section(
                        b, g, it, bt[b][0], bt[b][1], bt[b][2], bt[b][3]))

            def proj(b, tt):
                scoped(f"proj{b}", lambda: emit_proj_chunk(b, tt, bt[b][3]))

            # Emission order = scheduler priority. Attention sections come
            # BEFORE the qkv/proj work of the same round so scores matmuls
            # win ties on the PE and the ACT engine never starves; qkv/proj
            # fill the PE slack. qkv(b,it+1) is emitted after attn(b,it);
            # qkv(b+1,0) rides in batch b's last (ACT-heaviest) round.
            qkv(0, 0)
            for it in range(NIT):
                attn(0, it)
                qkv(0, it + 1) if it < 3 else qkv(1, 0)
            for it in range(NIT):
                attn(1, it)
                if it < 3:
                    qkv(1, it + 1)
                proj(0, it)
                if it > 0:
                    proj(1, it - 1)
            proj(1, 3)

    nc.compile()
    return nc


def _get_nc():
    global _NC
    if _NC is None:
        _NC = _build()
    return _NC


def kernel(x, w_qkv, b_qkv, w_proj, b_proj):
    global LAST_RESULT
    import ml_dtypes
    from concourse.bass_utils import run_bass_kernel_spmd

    bf16 = ml_dtypes.bfloat16
    x = np.asarray(x, dtype=np.float32)
    w_qkv = np.asarray(w_qkv, dtype=np.float32)
    b_qkv = np.asarray(b_qkv, dtype=np.float32)
    w_proj = np.asarray(w_proj, dtype=np.float32)
    b_proj = np.asarray(b_proj, dtype=np.float32)

    xTb = np.ascontiguousarray(x.transpose(0, 2, 1)).astype(bf16)  # [B, D, S]

    jj = np.arange(128)[:, None]
    ii = np.arange(128)[None, :]
    mask = (jj <= ii).astype(bf16)  # triangle band for diagonal blocks

    in_maps = []
    for c in range(8):
        bp, hg = c // 4, c % 4
        cols = slice(hg * HCOLS, (hg + 1) * HCOLS)
        w_q = w_qkv[:, 0:D][:, cols]
        w_k = w_qkv[:, D : 2 * D][:, cols]
        w_v = w_qkv[:, 2 * D : 3 * D][:, cols]
        in_maps.append({
            "xT": np.ascontiguousarray(xTb[2 * bp : 2 * bp + 2]),
            "wqk": np.concatenate([w_q, w_k], axis=1).astype(bf16),
            "wv": np.ascontiguousarray(w_v).astype(bf16),
            "wo": np.ascontiguousarray(w_proj[cols, :]).astype(bf16),
            "bqk": np.ascontiguousarray(
                np.concatenate([b_qkv[0:D][cols], b_qkv[D : 2 * D][cols]])
            ),
            "bv": np.ascontiguousarray(b_qkv[2 * D : 3 * D][cols]),
            "mask": mask,
        })

    nc = _get_nc()
    res = run_bass_kernel_spmd(nc, in_maps, core_ids=list(range(8)))
    LAST_RESULT = res

    out = np.zeros((B, S, D), dtype=np.float32)
    for c in range(8):
        bp = c // 4
        yTc = res.results[c]["yT"]  # [2, D, S] bf16 partial
        for bi in range(2):
            out[2 * bp + bi] += np.asarray(yTc[bi], dtype=np.float32).T
    out += b_proj[None, None, :]
    return out


# revision 16
# speedup vs baseline: 1.0287x; 1.0204x over previous
"""Causal self-attention on 8 TRN2 NeuronCores (Bass/Tile, bf16).

Sharding: core c = 4*bp + hg handles batches [2bp, 2bp+1] and heads
[4hg, 4hg+4). Host transposes x to [B, D, S] bf16, slices weights per
head group; cores emit y^T [D, S] bf16 partials which the host sums
over the 4 head groups and transposes.

Per-core kernel, per batch (phases FUSED chunk-wise so the ACT engine
never idles during QKV and the PE never idles during attention):
  for ch in 0..3:
    A) QKV projection for token chunk ch: q/k in pair-packed
       [2x64 hd, token] layout (2 heads per 128-partition tile), v in
       [token, hd+1] layout with a ones column (softmax denominators
       fall out of the AV matmul).
    B) attention sections (pair g, i-tile it=ch) - these need only
       k/v chunks <= ch. Per section the j-loop computes scores^T for
       BOTH heads concurrently via row-tiled K=64 matmuls
       (tile_position (0,0)/(64,0)), one exp over [128, 2*512] on ACT,
       a triangle mask multiply on the 128-col diagonal band only, and
       per-head AV accumulation with causal column trimming; AV(jt) is
       emitted after scores(jt+1) (1-block software pipeline skew).
    C) output projection chunk of the PREVIOUS batch (wo stationary).
PSUM: scores 2x2 banks, AV accumulators 2x1, qkv/proj aux 2x1 = 8.
"""
import numpy as np

B, S, D, H = 4, 2048, 1024, 16
HD = D // H            # 64
SCALE = 1.0 / np.sqrt(HD)
NB = 2                 # batches per core
NHC = 4                # heads per core (2 pairs)
HCOLS = NHC * HD       # 256 q/k/v columns per core
NDT = D // 128         # 8 D-tiles
NIT = S // 512         # 4 i-tiles per batch
NJT = S // 128         # 16 j-tiles per batch

_NC = None
LAST_RESULT = None


def _build():
    import concourse.bacc as bacc
    import concourse.mybir as mybir
    import concourse.tile as tile
    import concourse.bass as bass

    f32 = mybir.dt.float32
    bf16 = mybir.dt.bfloat16
    Act = mybir.ActivationFunctionType

    nc = bacc.Bacc(trn_type="TRN2", target_bir_lowering=False)
    xT = nc.dram_tensor("xT", [NB, D, S], bf16, kind="ExternalInput")
    wqk = nc.dram_tensor("wqk", [D, 2 * HCOLS], bf16, kind="ExternalInput")
    wv = nc.dram_tensor("wv", [D, HCOLS], bf16, kind="ExternalInput")
    wo = nc.dram_tensor("wo", [HCOLS, D], bf16, kind="ExternalInput")
    bqk = nc.dram_tensor("bqk", [2 * HCOLS], f32, kind="ExternalInput")
    bv = nc.dram_tensor("bv", [HCOLS], f32, kind="ExternalInput")
    mask = nc.dram_tensor("mask", [128, 128], bf16, kind="ExternalInput")
    yT = nc.dram_tensor("yT", [NB, D, S], bf16, kind="ExternalOutput")

    with tile.TileContext(nc) as tc:
        with (
            tc.tile_pool(name="singles", bufs=1) as singles,
            tc.tile_pool(name="xtp", bufs=2) as xtp,
            tc.tile_pool(name="qkp", bufs=2) as qkp,
            tc.tile_pool(name="vp", bufs=2) as vp,
            tc.tile_pool(name="attp", bufs=6) as attp,
            tc.tile_pool(name="yhp", bufs=2) as yhp,
            tc.tile_pool(name="stgp", bufs=6) as stgp,
            tc.tile_pool(name="bcp", bufs=2) as bcp,
            tc.tile_pool(name="pkp", bufs=2) as pkp,
            tc.tile_pool(name="outp", bufs=4) as outp,
            tc.tile_pool(name="dscr", bufs=2, space="DRAM") as dscr,
            tc.tile_pool(name="psS", bufs=2, space="PSUM") as psS,
            tc.tile_pool(name="psA", bufs=2, space="PSUM") as psA,
            tc.tile_pool(name="psX", bufs=2, space="PSUM") as psX,
        ):
            # ---- one-time loads ----
            # Startup-latency-critical loads: interleave wqk dt-slices with
            # the first x chunk's dt-slices on the SP queue so matmul dt=0
            # can begin ~2us in; everything else rides the ACT hardware-DGE
            # queue in parallel.
            wqk_sb = singles.tile([128, NDT, 2 * HCOLS], bf16, tag="wqk")
            wqk_r = wqk.ap().rearrange("(dt p) c -> p dt c", p=128)
            xt0 = xtp.tile([128, NDT, 512], bf16, tag="xt", name="xt_0_0")
            for dt in range(NDT):
                nc.sync.dma_start(out=wqk_sb[:, dt, :], in_=wqk_r[:, dt, :])
                nc.sync.dma_start(
                    out=xt0[:, dt, :],
                    in_=xT.ap()[0, dt * 128 : (dt + 1) * 128, 0:512],
                )
            wv_sb = singles.tile([128, NDT, HCOLS], bf16, tag="wv")
            nc.scalar.dma_start(
                out=wv_sb, in_=wv.ap().rearrange("(dt p) c -> p dt c", p=128)
            )
            bqk_sb = singles.tile([128, 4], f32, tag="bqk")
            nc.scalar.dma_start(
                out=bqk_sb, in_=bqk.ap().rearrange("(cb p) -> p cb", p=128)
            )
            mask_sb = singles.tile([128, 128], bf16, tag="mask")
            nc.scalar.dma_start(out=mask_sb, in_=mask.ap())
            wo_sb = singles.tile([128, 2, D], bf16, tag="wo")
            nc.scalar.dma_start(
                out=wo_sb, in_=wo.ap().rearrange("(kt p) c -> p kt c", p=128)
            )
            bv_sb = singles.tile([128, HCOLS], f32, tag="bv")
            bv_ap = bv.ap()
            nc.gpsimd.dma_start(
                out=bv_sb,
                in_=bass.AP(
                    tensor=bv_ap.tensor, offset=bv_ap.offset,
                    ap=[[0, 128], *bv_ap.ap],
                ),
            )
            ones_sb = singles.tile([128, NJT * NHC], bf16, tag="ones")
            nc.vector.memset(ones_sb[:], 1.0)

            def scoped(name, fn):
                s = nc.enter_named_scope(name, False)
                fn()
                nc.leave_named_scope(name, s[0], False)

            def emit_qkv_chunk(b, ch, qp, kT, v_sb):
                t0 = ch * 512
                if b == 0 and ch == 0:
                    xt = xt0  # loaded during startup
                else:
                    xt = xtp.tile([128, NDT, 512], bf16, tag="xt")
                    nc.sync.dma_start(
                        out=xt[:],
                        in_=xT.ap()[b, :, t0 : t0 + 512].rearrange(
                            "(dt p) s -> p dt s", p=128
                        ),
                    )
                for cb in range(4):  # q-pair0 q-pair1 k-pair0 k-pair1
                    ps = psX.tile([128, 512], f32, tag="aux",
                                  name=f"psqk_{b}_{ch}_{cb}")
                    for dt in range(NDT):
                        nc.tensor.matmul(
                            ps[:],
                            wqk_sb[:, dt, cb * 128 : (cb + 1) * 128],
                            xt[:, dt, :],
                            start=(dt == 0), stop=(dt == NDT - 1),
                        )
                    dst = qp[cb] if cb < 2 else kT[cb - 2]
                    nc.vector.tensor_scalar_add(
                        out=dst[:, t0 : t0 + 512],
                        in0=ps[:],
                        scalar1=bqk_sb[:, cb : cb + 1],
                    )
                for st in range(4):  # 128-token tiles for v
                    psv = psX.tile([128, 512], f32, tag="aux",
                                   name=f"psv_{b}_{ch}_{st}")
                    for dt in range(NDT):
                        nc.tensor.matmul(
                            psv[:, 0:HCOLS],
                            xt[:, dt, st * 128 : (st + 1) * 128],
                            wv_sb[:, dt, :],
                            start=(dt == 0), stop=(dt == NDT - 1),
                        )
                    nc.vector.tensor_add(
                        v_sb[:, ch * 4 + st, :, 0:HD],
                        psv[:, 0:HCOLS].rearrange("p (h c) -> p h c", h=NHC),
                        bv_sb[:].rearrange("p (h c) -> p h c", h=NHC),
                    )

            def emit_attn_section(b, g, it, qp, kT, v_sb, yh):
                acc = [
                    psA.tile([HD + 1, 512], f32, tag="acc",
                             name=f"acc_{b}_{g}_{it}_{hh}")
                    for hh in range(2)
                ]
                n_jt = 4 * it + 4
                atts = {}

                def emit_scores(jt):
                    diag = jt >= 4 * it
                    r = jt - 4 * it if diag else 0
                    w = 512 - 128 * r
                    pss = psS.tile([128, 2, 512], f32, tag="sc",
                                   name=f"pss_{b}_{g}_{it}_{jt}")
                    for hh in range(2):  # concurrent row-tiled pair
                        nc.tensor.matmul(
                            pss[:, hh, 128 * r : 512],
                            kT[g][64 * hh : 64 * hh + 64,
                                  jt * 128 : (jt + 1) * 128],
                            qp[g][64 * hh : 64 * hh + 64,
                                  it * 512 + 128 * r : (it + 1) * 512],
                            start=True, stop=True,
                        )
                    att = attp.tile([128, 2, 512], bf16, tag="att")
                    nc.scalar.activation(
                        out=att[:, :, 0:w],
                        in_=pss[:, :, 128 * r : 512],
                        func=Act.Exp, bias=0.0, scale=float(SCALE),
                    )
                    if diag:
                        for hh in range(2):
                            nc.vector.tensor_mul(
                                att[:, hh, 0:128], att[:, hh, 0:128],
                                mask_sb[:],
                            )
                    atts[jt] = att

                def emit_av(jt):
                    diag = jt >= 4 * it
                    r = jt - 4 * it if diag else 0
                    w = 512 - 128 * r
                    att = atts.pop(jt)
                    for hh in range(2):
                        nc.tensor.matmul(
                            acc[hh][:, 128 * r : 512],
                            v_sb[:, jt, 2 * g + hh, :],
                            att[:, hh, 0:w],
                            start=(jt == 0), stop=(jt == n_jt - 1),
                        )

                # 1-block software-pipeline skew
                for jt in range(n_jt + 1):
                    if jt < n_jt:
                        emit_scores(jt)
                    if jt >= 1:
                        emit_av(jt - 1)

                # section drain: denominators + stage + normalize
                dh = dscr.tile([2, 512], bf16, tag="dh",
                               name=f"dh_{b}_{g}_{it}")
                stgs = []
                for hh in range(2):
                    st_t = stgp.tile([HD + 1, 512], bf16, tag="stg",
                                     name=f"stg_{b}_{g}_{it}_{hh}")
                    nc.vector.tensor_copy(st_t[:], acc[hh][:])
                    nc.sync.dma_start(out=dh[hh, :], in_=st_t[HD : HD + 1, :])
                    stgs.append(st_t)
                pk = pkp.tile([128, 8], bf16, tag="pk", name=f"pk_{b}_{g}_{it}")
                nc.sync.dma_start(
                    out=pk[:],
                    in_=bass.AP(tensor=dh.tensor, offset=dh.offset,
                                ap=[[8, 128], [1, 8]]),
                )
                rec = pkp.tile([128, 8], bf16, tag="rec",
                               name=f"rec_{b}_{g}_{it}")
                with nc.allow_low_precision(
                    reason="bf16 softmax denominators within 2e-2 tolerance"
                ):
                    nc.vector.reciprocal(rec[:], pk[:])
                drec = dscr.tile([2, 512], bf16, tag="drec",
                                 name=f"drec_{b}_{g}_{it}")
                nc.sync.dma_start(
                    out=bass.AP(tensor=drec.tensor, offset=drec.offset,
                                ap=[[8, 128], [1, 8]]),
                    in_=rec[:],
                )
                bch = bcp.tile([64, 2, 512], bf16, tag="bch",
                               name=f"bch_{b}_{g}_{it}")
                nc.gpsimd.dma_start(
                    out=bch[:],
                    in_=bass.AP(tensor=drec.tensor, offset=drec.offset,
                                ap=[[0, 64], [1, 1024]]),
                )
                for hh in range(2):
                    nc.vector.tensor_mul(
                        yh[g][64 * hh : 64 * hh + 64,
                              it * 512 : (it + 1) * 512],
                        stgs[hh][0:HD, :],
                        bch[:, hh, :],
                    )

            def emit_proj_chunk(b, tt, yh):
                for dc in range(NDT):
                    pso = psX.tile([128, 512], f32, tag="aux",
                                   name=f"pso_{b}_{tt}_{dc}")
                    for kt in range(2):
                        nc.tensor.matmul(
                            pso[:],
                            wo_sb[:, kt, dc * 128 : (dc + 1) * 128],
                            yh[kt][:, tt * 512 : (tt + 1) * 512],
                            start=(kt == 0), stop=(kt == 1),
                        )
                    yo = outp.tile([128, 512], bf16, tag="yo")
                    nc.vector.tensor_copy(yo[:], pso[:])
                    nc.gpsimd.dma_start(
                        out=yT.ap()[b, dc * 128 : (dc + 1) * 128,
                                    tt * 512 : (tt + 1) * 512],
                        in_=yo[:],
                    )

            # Per-batch tiles for both batches up front.
            bt = []
            for b in range(NB):
                qp = [
                    qkp.tile([128, S], bf16, tag=f"qp{g}", name=f"qp{g}_{b}")
                    for g in range(2)
                ]
                kT = [
                    qkp.tile([128, S], bf16, tag=f"kT{g}", name=f"kT{g}_{b}")
                    for g in range(2)
                ]
                v_sb = vp.tile([128, NJT, NHC, HD + 1], bf16, tag="v",
                               name=f"v_{b}")
                yh = [
                    yhp.tile([128, S], bf16, tag=f"yh{g}", name=f"yh{g}_{b}")
                    for g in range(2)
                ]
                bt.append((qp, kT, v_sb, yh))

            def emit_ones(b):
                nc.vector.tensor_copy(
                    bt[b][2][:, :, :, HD : HD + 1],
                    ones_sb[:].rearrange("p (a b c) -> p a b c", a=NJT, b=NHC),
                )

            def qkv(b, ch):
                if ch == 0:
                    emit_ones(b)
                scoped(f"qkv{b}", lambda: emit_qkv_chunk(
                    b, ch, bt[b][0], bt[b][1], bt[b][2]))

            def attn(b, it):
                for g in range(2):
                    scoped(f"attn{b}", lambda g=g: emit_attn_section(
                        b, g, it, bt[b][0], bt[b][1], bt[b][2], bt[b][3]))

            def proj(b, tt):
                scoped(f"proj{b}", lambda: emit_proj_chunk(b, tt, bt[b][3]))

            # Emission order = scheduler priority. Attention sections come
            # BEFORE the qkv/proj work of the same round so scores matmuls
            # win ties on the PE and the ACT engine never starves; qkv/proj
            # fill the PE slack. qkv(b,it+1) is emitted after attn(b,it);
            # qkv(b+1,0) rides in batch b's last (ACT-heaviest) round.
            qkv(0, 0)
            for it in range(NIT):
                attn(0, it)
                qkv(0, it + 1) if it < 3 else qkv(1, 0)
            for it in range(NIT):
                attn(1, it)
                if it < 3:
                    qkv(1, it + 1)
                proj(0, it)
                if it > 0:
                    proj(1, it - 1)
            proj(1, 3)

    nc.compile()
    return nc


def _get_nc():
    global _NC
    if _NC is None:
        _NC = _build()
    return _NC


def kernel(x, w_qkv, b_qkv, w_proj, b_proj):
    global LAST_RESULT
    import ml_dtypes
    from concourse.bass_utils import run_bass_kernel_spmd

    bf16 = ml_dtypes.bfloat16
    x = np.asarray(x, dtype=np.float32)
    w_qkv = np.asarray(w_qkv, dtype=np.float32)
    b_qkv = np.asarray(b_qkv, dtype=np.float32)
    w_proj = np.asarray(w_proj, dtype=np.float32)
    b_proj = np.asarray(b_proj, dtype=np.float32)

    xTb = np.ascontiguousarray(x.transpose(0, 2, 1)).astype(bf16)  # [B, D, S]

    jj = np.arange(128)[:, None]
    ii = np.arange(128)[None, :]
    mask = (jj <= ii).astype(bf16)  # triangle band for diagonal blocks

    in_maps = []
    for c in range(8):
        bp, hg = c // 4, c % 4
        cols = slice(hg * HCOLS, (hg + 1) * HCOLS)
        w_q = w_qkv[:, 0:D][:, cols]
        w_k = w_qkv[:, D : 2 * D][:, cols]
        w_v = w_qkv[:, 2 * D : 3 * D][:, cols]
        in_maps.append({
            "xT": np.ascontiguousarray(xTb[2 * bp : 2 * bp + 2]),
            "wqk": np.concatenate([w_q, w_k], axis=1).astype(bf16),
            "wv": np.ascontiguousarray(w_v).astype(bf16),
            "wo": np.ascontiguousarray(w_proj[cols, :]).astype(bf16),
            "bqk": np.ascontiguousarray(
                np.concatenate([b_qkv[0:D][cols], b_qkv[D : 2 * D][cols]])
            ),
            "bv": np.ascontiguousarray(b_qkv[2 * D : 3 * D][cols]),
            "mask": mask,
        })

    nc = _get_nc()
    res = run_bass_kernel_spmd(nc, in_maps, core_ids=list(range(8)))
    LAST_RESULT = res

    out = np.zeros((B, S, D), dtype=np.float32)
    for c in range(8):
        bp = c // 4
        yTc = res.results[c]["yT"]  # [2, D, S] bf16 partial
        for bi in range(2):
            out[2 * bp + bi] += np.asarray(yTc[bi], dtype=np.float32).T
    out += b_proj[None, None, :]
    return out


# revision 17
# speedup vs baseline: 1.0600x; 1.0304x over previous
"""Causal self-attention on 8 TRN2 NeuronCores (Bass/Tile, bf16).

Sharding: core c = 4*bp + hg handles batches [2bp, 2bp+1] and heads
[4hg, 4hg+4). Host transposes x to [B, D, S] bf16, slices weights per
head group; cores emit y^T [D, S] bf16 partials which the host sums
over the 4 head groups and transposes.

Per-core kernel, per batch (phases FUSED chunk-wise so the ACT engine
never idles during QKV and the PE never idles during attention):
  for ch in 0..3:
    A) QKV projection for token chunk ch: q/k in pair-packed
       [2x64 hd, token] layout (2 heads per 128-partition tile), v in
       [token, hd+1] layout with a ones column (softmax denominators
       fall out of the AV matmul).
    B) attention sections (pair g, i-tile it=ch) - these need only
       k/v chunks <= ch. Per section the j-loop computes scores^T for
       BOTH heads concurrently via row-tiled K=64 matmuls
       (tile_position (0,0)/(64,0)), one exp over [128, 2*512] on ACT,
       a triangle mask multiply on the 128-col diagonal band only, and
       per-head AV accumulation with causal column trimming; AV(jt) is
       emitted after scores(jt+1) (1-block software pipeline skew).
    C) output projection chunk of the PREVIOUS batch (wo stationary).
PSUM: scores 2x2 banks, AV accumulators 2x1, qkv/proj aux 2x1 = 8.
"""
import numpy as np

B, S, D, H = 4, 2048, 1024, 16
HD = D // H            # 64
SCALE = 1.0 / np.sqrt(HD)
NB = 2                 # batches per core
NHC = 4                # heads per core (2 pairs)
HCOLS = NHC * HD       # 256 q/k/v columns per core
NDT = D // 128         # 8 D-tiles
NIT = S // 512         # 4 i-tiles per batch
NJT = S // 128         # 16 j-tiles per batch

_NC = None
LAST_RESULT = None


def _build():
    import concourse.bacc as bacc
    import concourse.mybir as mybir
    import concourse.tile as tile
    import concourse.bass as bass

    f32 = mybir.dt.float32
    bf16 = mybir.dt.bfloat16
    Act = mybir.ActivationFunctionType

    nc = bacc.Bacc(trn_type="TRN2", target_bir_lowering=False)
    xT = nc.dram_tensor("xT", [NB, D, S], bf16, kind="ExternalInput")
    wqk = nc.dram_tensor("wqk", [D, 2 * HCOLS], bf16, kind="ExternalInput")
    wv = nc.dram_tensor("wv", [D, HCOLS], bf16, kind="ExternalInput")
    wo = nc.dram_tensor("wo", [HCOLS, D], bf16, kind="ExternalInput")
    bqk = nc.dram_tensor("bqk", [2 * HCOLS], f32, kind="ExternalInput")
    bv = nc.dram_tensor("bv", [HCOLS], f32, kind="ExternalInput")
    mask = nc.dram_tensor("mask", [128, 128], bf16, kind="ExternalInput")
    yT = nc.dram_tensor("yT", [NB, D, S], bf16, kind="ExternalOutput")

    with tile.TileContext(nc) as tc:
        with (
            tc.tile_pool(name="singles", bufs=1) as singles,
            tc.tile_pool(name="xtp", bufs=2) as xtp,
            tc.tile_pool(name="qkp", bufs=2) as qkp,
            tc.tile_pool(name="vp", bufs=2) as vp,
            tc.tile_pool(name="attp", bufs=6) as attp,
            tc.tile_pool(name="yhp", bufs=2) as yhp,
            tc.tile_pool(name="stgp", bufs=6) as stgp,
            tc.tile_pool(name="bcp", bufs=2) as bcp,
            tc.tile_pool(name="pkp", bufs=2) as pkp,
            tc.tile_pool(name="outp", bufs=4) as outp,
            tc.tile_pool(name="dscr", bufs=2, space="DRAM") as dscr,
            tc.tile_pool(name="psS", bufs=2, space="PSUM") as psS,
            tc.tile_pool(name="psA", bufs=2, space="PSUM") as psA,
            tc.tile_pool(name="psX", bufs=2, space="PSUM") as psX,
        ):
            # ---- one-time loads ----
            # Startup-latency-critical loads: interleave wqk dt-slices with
            # the first x chunk's dt-slices on the SP queue so matmul dt=0
            # can begin ~2us in; everything else rides the ACT hardware-DGE
            # queue in parallel.
            wqk_sb = singles.tile([128, NDT, 2 * HCOLS], bf16, tag="wqk")
            wqk_r = wqk.ap().rearrange("(dt p) c -> p dt c", p=128)
            xt0 = xtp.tile([128, NDT, 512], bf16, tag="xt", name="xt_0_0")
            for dt in range(NDT):
                nc.sync.dma_start(out=wqk_sb[:, dt, :], in_=wqk_r[:, dt, :])
                nc.sync.dma_start(
                    out=xt0[:, dt, :],
                    in_=xT.ap()[0, dt * 128 : (dt + 1) * 128, 0:512],
                )
            wv_sb = singles.tile([128, NDT, HCOLS], bf16, tag="wv")
            nc.scalar.dma_start(
                out=wv_sb, in_=wv.ap().rearrange("(dt p) c -> p dt c", p=128)
            )
            bqk_sb = singles.tile([128, 4], f32, tag="bqk")
            nc.scalar.dma_start(
                out=bqk_sb, in_=bqk.ap().rearrange("(cb p) -> p cb", p=128)
            )
            mask_sb = singles.tile([128, 128], bf16, tag="mask")
            nc.scalar.dma_start(out=mask_sb, in_=mask.ap())
            wo_sb = singles.tile([128, 2, D], bf16, tag="wo")
            nc.scalar.dma_start(
                out=wo_sb, in_=wo.ap().rearrange("(kt p) c -> p kt c", p=128)
            )
            bv_sb = singles.tile([128, HCOLS], f32, tag="bv")
            bv_ap = bv.ap()
            nc.gpsimd.dma_start(
                out=bv_sb,
                in_=bass.AP(
                    tensor=bv_ap.tensor, offset=bv_ap.offset,
                    ap=[[0, 128], *bv_ap.ap],
                ),
            )
            ones_sb = singles.tile([128, NJT * NHC], bf16, tag="ones")
            nc.vector.memset(ones_sb[:], 1.0)

            def scoped(name, fn):
                s = nc.enter_named_scope(name, False)
                fn()
                nc.leave_named_scope(name, s[0], False)

            def emit_qkv_chunk(b, ch, qp, kT, v_sb):
                t0 = ch * 512
                if b == 0 and ch == 0:
                    xt = xt0  # loaded during startup
                else:
                    xt = xtp.tile([128, NDT, 512], bf16, tag="xt")
                    nc.sync.dma_start(
                        out=xt[:],
                        in_=xT.ap()[b, :, t0 : t0 + 512].rearrange(
                            "(dt p) s -> p dt s", p=128
                        ),
                    )
                for cb in range(4):  # q-pair0 q-pair1 k-pair0 k-pair1
                    ps = psX.tile([128, 512], f32, tag="aux",
                                  name=f"psqk_{b}_{ch}_{cb}")
                    for dt in range(NDT):
                        nc.tensor.matmul(
                            ps[:],
                            wqk_sb[:, dt, cb * 128 : (cb + 1) * 128],
                            xt[:, dt, :],
                            start=(dt == 0), stop=(dt == NDT - 1),
                        )
                    dst = qp[cb] if cb < 2 else kT[cb - 2]
                    nc.vector.tensor_scalar_add(
                        out=dst[:, t0 : t0 + 512],
                        in0=ps[:],
                        scalar1=bqk_sb[:, cb : cb + 1],
                    )
                for st in range(4):  # 128-token tiles for v
                    psv = psX.tile([128, 512], f32, tag="aux",
                                   name=f"psv_{b}_{ch}_{st}")
                    for dt in range(NDT):
                        nc.tensor.matmul(
                            psv[:, 0:HCOLS],
                            xt[:, dt, st * 128 : (st + 1) * 128],
                            wv_sb[:, dt, :],
                            start=(dt == 0), stop=(dt == NDT - 1),
                        )
                    nc.vector.tensor_add(
                        v_sb[:, ch * 4 + st, :, 0:HD],
                        psv[:, 0:HCOLS].rearrange("p (h c) -> p h c", h=NHC),
                        bv_sb[:].rearrange("p (h c) -> p h c", h=NHC),
                    )

            def emit_attn_section(b, g, it, qp, kT, v_sb, yh):
                acc = [
                    psA.tile([HD + 1, 512], f32, tag="acc",
                             name=f"acc_{b}_{g}_{it}_{hh}")
                    for hh in range(2)
                ]
                n_jt = 4 * it + 4
                atts = {}

                def emit_scores(jt):
                    diag = jt >= 4 * it
                    r = jt - 4 * it if diag else 0
                    w = 512 - 128 * r
                    pss = psS.tile([128, 2, 512], f32, tag="sc",
                                   name=f"pss_{b}_{g}_{it}_{jt}")
                    for hh in range(2):  # concurrent row-tiled pair
                        nc.tensor.matmul(
                            pss[:, hh, 128 * r : 512],
                            kT[g][64 * hh : 64 * hh + 64,
                                  jt * 128 : (jt + 1) * 128],
                            qp[g][64 * hh : 64 * hh + 64,
                                  it * 512 + 128 * r : (it + 1) * 512],
                            start=True, stop=True,
                        )
                    att = attp.tile([128, 2, 512], bf16, tag="att")
                    nc.scalar.activation(
                        out=att[:, :, 0:w],
                        in_=pss[:, :, 128 * r : 512],
                        func=Act.Exp, bias=0.0, scale=float(SCALE),
                    )
                    if diag:
                        for hh in range(2):
                            nc.vector.tensor_mul(
                                att[:, hh, 0:128], att[:, hh, 0:128],
                                mask_sb[:],
                            )
                    atts[jt] = att

                def emit_av(jt):
                    diag = jt >= 4 * it
                    r = jt - 4 * it if diag else 0
                    w = 512 - 128 * r
                    att = atts.pop(jt)
                    for hh in range(2):
                        nc.tensor.matmul(
                            acc[hh][:, 128 * r : 512],
                            v_sb[:, jt, 2 * g + hh, :],
                            att[:, hh, 0:w],
                            start=(jt == 0), stop=(jt == n_jt - 1),
                        )

                # 1-block software-pipeline skew
                for jt in range(n_jt + 1):
                    if jt < n_jt:
                        emit_scores(jt)
                    if jt >= 1:
                        emit_av(jt - 1)

                # section drain: denominators + stage + normalize
                stgs = []
                pk = pkp.tile([128, 8], bf16, tag="pk", name=f"pk_{b}_{g}_{it}")
                for hh in range(2):
                    st_t = stgp.tile([HD + 1, 512], bf16, tag="stg",
                                     name=f"stg_{b}_{g}_{it}_{hh}")
                    nc.vector.tensor_copy(st_t[:], acc[hh][:])
                    # SBUF->SBUF cross-partition pack: den row [1,512] ->
                    # [64,8] so the reciprocal runs 128 lanes wide
                    nc.sync.dma_start(
                        out=pk[64 * hh : 64 * hh + 64, :],
                        in_=st_t[HD : HD + 1, :],
                    )
                    stgs.append(st_t)
                rec = pkp.tile([128, 8], bf16, tag="rec",
                               name=f"rec_{b}_{g}_{it}")
                with nc.allow_low_precision(
                    reason="bf16 softmax denominators within 2e-2 tolerance"
                ):
                    nc.vector.reciprocal(rec[:], pk[:])
                drec = dscr.tile([2, 512], bf16, tag="drec",
                                 name=f"drec_{b}_{g}_{it}")
                nc.sync.dma_start(
                    out=bass.AP(tensor=drec.tensor, offset=drec.offset,
                                ap=[[8, 128], [1, 8]]),
                    in_=rec[:],
                )
                bch = bcp.tile([64, 2, 512], bf16, tag="bch",
                               name=f"bch_{b}_{g}_{it}")
                nc.gpsimd.dma_start(
                    out=bch[:],
                    in_=bass.AP(tensor=drec.tensor, offset=drec.offset,
                                ap=[[0, 64], [1, 1024]]),
                )
                for hh in range(2):
                    nc.vector.tensor_mul(
                        yh[g][64 * hh : 64 * hh + 64,
                              it * 512 : (it + 1) * 512],
                        stgs[hh][0:HD, :],
                        bch[:, hh, :],
                    )

            def emit_proj_chunk(b, tt, yh):
                for dc in range(NDT):
                    pso = psX.tile([128, 512], f32, tag="aux",
                                   name=f"pso_{b}_{tt}_{dc}")
                    for kt in range(2):
                        nc.tensor.matmul(
                            pso[:],
                            wo_sb[:, kt, dc * 128 : (dc + 1) * 128],
                            yh[kt][:, tt * 512 : (tt + 1) * 512],
                            start=(kt == 0), stop=(kt == 1),
                        )
                    yo = outp.tile([128, 512], bf16, tag="yo")
                    nc.vector.tensor_copy(yo[:], pso[:])
                    nc.gpsimd.dma_start(
                        out=yT.ap()[b, dc * 128 : (dc + 1) * 128,
                                    tt * 512 : (tt + 1) * 512],
                        in_=yo[:],
                    )

            # Per-batch tiles for both batches up front.
            bt = []
            for b in range(NB):
                qp = [
                    qkp.tile([128, S], bf16, tag=f"qp{g}", name=f"qp{g}_{b}")
                    for g in range(2)
                ]
                kT = [
                    qkp.tile([128, S], bf16, tag=f"kT{g}", name=f"kT{g}_{b}")
                    for g in range(2)
                ]
                v_sb = vp.tile([128, NJT, NHC, HD + 1], bf16, tag="v",
                               name=f"v_{b}")
                yh = [
                    yhp.tile([128, S], bf16, tag=f"yh{g}", name=f"yh{g}_{b}")
                    for g in range(2)
                ]
                bt.append((qp, kT, v_sb, yh))

            def emit_ones(b):
                nc.vector.tensor_copy(
                    bt[b][2][:, :, :, HD : HD + 1],
                    ones_sb[:].rearrange("p (a b c) -> p a b c", a=NJT, b=NHC),
                )

            def qkv(b, ch):
                if ch == 0:
                    emit_ones(b)
                scoped(f"qkv{b}", lambda: emit_qkv_chunk(
                    b, ch, bt[b][0], bt[b][1], bt[b][2]))

            def attn(b, it):
                for g in range(2):
                    scoped(f"attn{b}", lambda g=g: emit_attn_section(
                        b, g, it, bt[b][0], bt[b][1], bt[b][2], bt[b][3]))

            def proj(b, tt):
                scoped(f"proj{b}", lambda: emit_proj_chunk(b, tt, bt[b][3]))

            # Emission order = scheduler priority. Attention sections come
            # BEFORE the qkv/proj work of the same round so scores matmuls
            # win ties on the PE and the ACT engine never starves; qkv/proj
            # fill the PE slack. qkv(b,it+1) is emitted after attn(b,it);
            # qkv(b+1,0) rides in batch b's last (ACT-heaviest) round.
            qkv(0, 0)
            for it in range(NIT):
                attn(0, it)
                qkv(0, it + 1) if it < 3 else qkv(1, 0)
            for it in range(NIT):
                attn(1, it)
                if it < 3:
                    qkv(1, it + 1)
                proj(0, it)
                if it > 0:
                    proj(1, it - 1)
            proj(1, 3)

    nc.compile()
    return nc


def _get_nc():
    global _NC
    if _NC is None:
        _NC = _build()
    return _NC


def kernel(x, w_qkv, b_qkv, w_proj, b_proj):
    global LAST_RESULT
    import ml_dtypes
    from concourse.bass_utils import run_bass_kernel_spmd

    bf16 = ml_dtypes.bfloat16
    x = np.asarray(x, dtype=np.float32)
    w_qkv = np.asarray(w_qkv, dtype=np.float32)
    b_qkv = np.asarray(b_qkv, dtype=np.float32)
    w_proj = np.asarray(w_proj, dtype=np.float32)
    b_proj = np.asarray(b_proj, dtype=np.float32)

    xTb = np.ascontiguousarray(x.transpose(0, 2, 1)).astype(bf16)  # [B, D, S]

    jj = np.arange(128)[:, None]
    ii = np.arange(128)[None, :]
    mask = (jj <= ii).astype(bf16)  # triangle band for diagonal blocks

    in_maps = []
    for c in range(8):
        bp, hg = c // 4, c % 4
        cols = slice(hg * HCOLS, (hg + 1) * HCOLS)
        w_q = w_qkv[:, 0:D][:, cols]
        w_k = w_qkv[:, D : 2 * D][:, cols]
        w_v = w_qkv[:, 2 * D : 3 * D][:, cols]
        in_maps.append({
            "xT": np.ascontiguousarray(xTb[2 * bp : 2 * bp + 2]),
            "wqk": np.concatenate([w_q, w_k], axis=1).astype(bf16),
            "wv": np.ascontiguousarray(w_v).astype(bf16),
            "wo": np.ascontiguousarray(w_proj[cols, :]).astype(bf16),
            "bqk": np.ascontiguousarray(
                np.concatenate([b_qkv[0:D][cols], b_qkv[D : 2 * D][cols]])
            ),
            "bv": np.ascontiguousarray(b_qkv[2 * D : 3 * D][cols]),
            "mask": mask,
        })

    nc = _get_nc()
    res = run_bass_kernel_spmd(nc, in_maps, core_ids=list(range(8)))
    LAST_RESULT = res

    out = np.zeros((B, S, D), dtype=np.float32)
    for c in range(8):
        bp = c // 4
        yTc = res.results[c]["yT"]  # [2, D, S] bf16 partial
        for bi in range(2):
            out[2 * bp + bi] += np.asarray(yTc[bi], dtype=np.float32).T
    out += b_proj[None, None, :]
    return out
